# revision 9
# baseline (speedup 1.0000x reference)
"""Multi-head attention (B=4, S=2048, D=1024, H=16) on 8 TRN2 NeuronCores.

Sharding: token-parallel, no collectives. Core c handles batch b=c//2,
query-token half h=c%2 (1024 tokens), all 16 heads. K/V projections for the
batch are recomputed by both cores of a pair (25% extra FLOPs) so no
cross-core communication is needed; the host only concatenates outputs.

Per-core dataflow (all matmuls fp32r, PSUM fp32):
  Q_T[o,t]  = WqT-chunks.T @ xqT   (+bq folded into the PSUM->SBUF copy)
  K_T[o,tk] = WkT-chunks.T @ xkT   (+bk likewise)
  V[tk,o]   = xvT-chunks.T @ WvT   (+bv via rank-1 matmul), packed 65-wide per
                                    head with a ones column for softmax sums
  S_T[tk,tq] = K_T_head.T @ Q_T_head   (two heads packed via PE row tiling,
                                        full tq=1024 in a 2-bank PSUM tile)
  E = exp(S_T/8 + mask_add[tk])        (one ACT op per [128,1024] tile,
                                        mask folded into the bias operand)
  ctx_aug[65,tq] = V_head_aug.T @ E    (row 64 = sum of exps = softmax denom)
  ctx_norm = ctx * bcast(1/denom)      (rank-1 matmul broadcast + DVE mul)
  out[tq,o] = ctx_norm-chunks.T @ WoT  (+bo via rank-1), DMA to DRAM

Q and ctx_norm are spilled to DRAM to stay inside the SBUF budget.
"""

import contextlib

import numpy as np

import concourse.bacc as bacc
import concourse.tile as tile
from concourse import mybir
from concourse.bass_utils import run_bass_kernel_spmd

B, S, D = 4, 2048, 1024
H, DH = 16, 64
NCORES = 8
TQ = 1024          # query tokens per core
NEG = -1000000000.0

F32 = mybir.dt.float32
F32R = mybir.dt.float32r
EXP = mybir.ActivationFunctionType.Exp

_CACHE = {}


def _build(loop_n=1, phases="abc23"):
    nc = bacc.Bacc("TRN2", target_bir_lowering=False)

    xqT = nc.dram_tensor("xqT", (D, TQ), F32R, kind="ExternalInput")
    xkT = nc.dram_tensor("xkT", (D, S), F32R, kind="ExternalInput")
    xvT = nc.dram_tensor("xvT", (D, S), F32R, kind="ExternalInput")
    wqT = nc.dram_tensor("wqT", (D, D), F32R, kind="ExternalInput")
    wkT = nc.dram_tensor("wkT", (D, D), F32R, kind="ExternalInput")
    wvT = nc.dram_tensor("wvT", (D, D), F32R, kind="ExternalInput")
    woT = nc.dram_tensor("woT", (D, D), F32R, kind="ExternalInput")
    bqcd = nc.dram_tensor("bq_col", (128, 8), F32, kind="ExternalInput")
    bkcd = nc.dram_tensor("bk_col", (128, 8), F32, kind="ExternalInput")
    bvd = nc.dram_tensor("bv", (1, D), F32R, kind="ExternalInput")
    bod = nc.dram_tensor("bo", (1, D), F32R, kind="ExternalInput")
    maskd = nc.dram_tensor("mask_add", (128, S // 128), F32, kind="ExternalInput")
    onesd = nc.dram_tensor("ones_row", (1, 512), F32R, kind="ExternalInput")
    onescold = nc.dram_tensor("ones_col", (128, S // 128), F32R, kind="ExternalInput")
    ctxd = nc.dram_tensor("ctx_spill", (D, TQ), F32R)  # internal scratch
    qd = nc.dram_tensor("q_spill", (D, TQ), F32R)      # internal scratch
    outd = nc.dram_tensor("out", (TQ, D), F32, kind="ExternalOutput")

    with tile.TileContext(nc, pool_alloc_mode="queue") as tc:
        loop = tc.For_i(0, loop_n, 1) if loop_n > 1 else contextlib.nullcontext()
        with loop, tc.tile_pool(name="const", bufs=1) as cp:
            ones = cp.tile([1, 512], F32R, tag="ones")
            nc.sync.dma_start(out=ones, in_=onesd.ap())
            maskc = cp.tile([128, S // 128], F32, tag="maskc")
            nc.sync.dma_start(out=maskc, in_=maskd.ap())

            # ---- Phase 1a: Q projection (out [o, tq]), spilled to DRAM ----
            if "a" in phases:
                with tc.tile_pool(name="xq", bufs=1) as xp, \
                     tc.tile_pool(name="wq", bufs=1) as wp, \
                     tc.tile_pool(name="qsb", bufs=4) as qsp, \
                     tc.tile_pool(name="ps1", bufs=4, space="PSUM") as ps:
                    xq = [xp.tile([128, TQ], F32R, name=f"xq{i}", tag=f"xq{i}")
                          for i in range(8)]
                    wq = [wp.tile([128, D], F32R, name=f"wq{i}", tag=f"wq{i}")
                          for i in range(8)]
                    bqc = xp.tile([128, 8], F32, tag="bqc")
                    nc.sync.dma_start(out=bqc, in_=bqcd.ap())
                    for i in range(8):
                        nc.sync.dma_start(out=xq[i],
                                          in_=xqT.ap()[i * 128:(i + 1) * 128, :])
                        nc.sync.dma_start(out=wq[i],
                                          in_=wqT.ap()[i * 128:(i + 1) * 128, :])
                    for oc in range(8):
                        for nb in range(TQ // 512):
                            p = ps.tile([128, 512], F32)
                            for ic in range(8):
                                nc.tensor.matmul(
                                    p, wq[ic][:, oc * 128:(oc + 1) * 128],
                                    xq[ic][:, nb * 512:(nb + 1) * 512],
                                    start=(ic == 0), stop=(ic == 7))
                            qsb = qsp.tile([128, 512], F32R, tag="qsb")
                            nc.vector.tensor_scalar_add(
                                out=qsb, in0=p, scalar1=bqc[:, oc:oc + 1])
                            nc.sync.dma_start(
                                out=qd.ap()[oc * 128:(oc + 1) * 128,
                                            nb * 512:(nb + 1) * 512], in_=qsb)

            # ---- Phase 1b: K projection (out [o, tk]) ----
            if "b" in phases:
                kp = tc.alloc_tile_pool(name="kpool", bufs=1)
                K = [kp.tile([128, S], F32R, name=f"k{i}", tag=f"k{i}")
                     for i in range(8)]
                with tc.tile_pool(name="xk", bufs=1) as xp, \
                     tc.tile_pool(name="wk", bufs=1) as wp, \
                     tc.tile_pool(name="ps1b", bufs=4, space="PSUM") as ps:
                    xk = [xp.tile([128, S], F32R, name=f"xk{i}", tag=f"xk{i}")
                          for i in range(8)]
                    wk = [wp.tile([128, D], F32R, name=f"wk{i}", tag=f"wk{i}")
                          for i in range(8)]
                    bkc = wp.tile([128, 8], F32, tag="bkc")
                    nc.sync.dma_start(out=bkc, in_=bkcd.ap())
                    for i in range(8):
                        nc.sync.dma_start(out=xk[i],
                                          in_=xkT.ap()[i * 128:(i + 1) * 128, :])
                        nc.sync.dma_start(out=wk[i],
                                          in_=wkT.ap()[i * 128:(i + 1) * 128, :])
                    for oc in range(8):
                        for nb in range(S // 512):
                            p = ps.tile([128, 512], F32)
                            for ic in range(8):
                                nc.tensor.matmul(
                                    p, wk[ic][:, oc * 128:(oc + 1) * 128],
                                    xk[ic][:, nb * 512:(nb + 1) * 512],
                                    start=(ic == 0), stop=(ic == 7))
                            nc.vector.tensor_scalar_add(
                                out=K[oc][:, nb * 512:(nb + 1) * 512], in0=p,
                                scalar1=bkc[:, oc:oc + 1])

            # ---- Phase 1c: V projection (out [tk, o], 65-packed + ones col) ----
            if "c" in phases:
                vp = tc.alloc_tile_pool(name="vpool", bufs=1)
                V = [vp.tile([128, H * 65], F32R, name=f"v{i}", tag=f"v{i}")
                     for i in range(16)]
                onescol_r = onescold.ap().rearrange("p (c o) -> p c o", o=1)
                for t in V:  # ones column per head for softmax denominators
                    vv = t[:].rearrange("p (h c) -> p h c", c=65)
                    nc.sync.dma_start(out=vv[:, :, 64:65], in_=onescol_r)
                xvT_r = xvT.ap().rearrange("(c p) t -> p c t", p=128)  # [128,8,S]
                with tc.tile_pool(name="xv", bufs=2) as xp, \
                     tc.tile_pool(name="wv", bufs=1) as wp, \
                     tc.tile_pool(name="ps1c", bufs=4, space="PSUM") as ps:
                    wv = [wp.tile([128, D], F32R, name=f"wv{i}", tag=f"wv{i}")
                          for i in range(8)]
                    bv = wp.tile([1, D], F32R, tag="bv")
                    nc.sync.dma_start(out=bv, in_=bvd.ap())
                    for i in range(8):
                        nc.sync.dma_start(out=wv[i],
                                          in_=wvT.ap()[i * 128:(i + 1) * 128, :])
                    for tk in range(16):
                        xvt_lo = xp.tile([128, 4, 128], F32R, tag="xvt_lo")
                        xvt_hi = xp.tile([128, 4, 128], F32R, tag="xvt_hi")
                        nc.sync.dma_start(
                            out=xvt_lo, in_=xvT_r[:, 0:4, tk * 128:(tk + 1) * 128])
                        nc.sync.dma_start(
                            out=xvt_hi, in_=xvT_r[:, 4:8, tk * 128:(tk + 1) * 128])
                        for nb in range(2):
                            p = ps.tile([128, 512], F32)
                            for ic in range(8):
                                src = xvt_lo if ic < 4 else xvt_hi
                                nc.tensor.matmul(
                                    p, src[:, ic % 4, :],
                                    wv[ic][:, nb * 512:(nb + 1) * 512],
                                    start=(ic == 0), stop=False)
                            nc.tensor.matmul(p, ones[:, 0:128],
                                             bv[:, nb * 512:(nb + 1) * 512],
                                             start=False, stop=True)
                            dst = V[tk][:].rearrange("p (h c) -> p h c", c=65)
                            srcp = p[:].rearrange("p (h c) -> p h c", c=64)
                            nc.vector.tensor_copy(
                                out=dst[:, 8 * nb:8 * nb + 8, 0:64], in_=srcp)

            # ---- Phase 2: attention per head-pair; ctx_norm spilled to DRAM ----
            if "2" in phases:
                with tc.tile_pool(name="expp", bufs=3) as ep, \
                     tc.tile_pool(name="smallp", bufs=3) as sp, \
                     tc.tile_pool(name="ctxout", bufs=4) as cop, \
                     tc.tile_pool(name="qin", bufs=2) as qip, \
                     tc.tile_pool(name="ps_s", bufs=1, space="PSUM") as ps_s, \
                     tc.tile_pool(name="ps_c", bufs=1, space="PSUM") as ps_c:
                    for hp in range(8):
                        qhp = qip.tile([128, TQ], F32R, tag="qhp")
                        nc.sync.dma_start(
                            out=qhp, in_=qd.ap()[hp * 128:(hp + 1) * 128, :])
                        # 4 live ctx accumulators: (headA/B) x (tq half 0/1)
                        cs = [ps_c.tile([65, 512], F32, name=f"c{j}", tag=f"c{j}")
                              for j in range(4)]
                        for tk in range(16):
                            ks = slice(tk * 128, (tk + 1) * 128)
                            sA = ps_s.tile([128, TQ], F32, tag="sA")
                            sB = ps_s.tile([128, TQ], F32, tag="sB")
                            for qb in range(2):
                                qs = slice(qb * 512, (qb + 1) * 512)
                                nc.tensor.matmul(sA[:, qs], K[hp][0:64, ks],
                                                 qhp[0:64, qs],
                                                 start=True, stop=True,
                                                 tile_position=(0, 0))
                                nc.tensor.matmul(sB[:, qs], K[hp][64:128, ks],
                                                 qhp[64:128, qs],
                                                 start=True, stop=True,
                                                 tile_position=(64, 0))
                            eA = ep.tile([128, TQ], F32R, tag="eA")
                            eB = ep.tile([128, TQ], F32R, tag="eB")
                            nc.scalar.activation(out=eA, in_=sA, func=EXP,
                                                 bias=maskc[:, tk:tk + 1],
                                                 scale=0.125)
                            nc.scalar.activation(out=eB, in_=sB, func=EXP,
                                                 bias=maskc[:, tk:tk + 1],
                                                 scale=0.125)
                            vA = V[tk][:, 130 * hp:130 * hp + 65]
                            vB = V[tk][:, 130 * hp + 65:130 * hp + 130]
                            for qb in range(2):
                                qs = slice(qb * 512, (qb + 1) * 512)
                                nc.tensor.matmul(cs[qb], vA, eA[:, qs],
                                                 start=(tk == 0), stop=(tk == 15))
                                nc.tensor.matmul(cs[2 + qb], vB, eB[:, qs],
                                                 start=(tk == 0), stop=(tk == 15))
                        for j, c_ps in enumerate(cs):
                            row0 = 64 * (j // 2)     # head A rows 0:64, B 64:128
                            qb = j % 2
                            qs = slice(qb * 512, (qb + 1) * 512)
                            rinv = sp.tile([1, 512], F32R, tag="rinv")
                            with nc.allow_low_precision(
                                    reason="f32r rounding of softmax denom"):
                                nc.vector.reciprocal(out=rinv, in_=c_ps[64:65, :])
                            b_ps = ps_s.tile([64, 512], F32, tag="sA")
                            nc.tensor.matmul(b_ps, ones[:, 0:64], rinv,
                                             start=True, stop=True)
                            bsb = sp.tile([64, 512], F32, tag="bsb")
                            nc.vector.tensor_copy(out=bsb, in_=b_ps)
                            cn = cop.tile([64, 512], F32R, tag="cn")
                            nc.vector.tensor_mul(out=cn, in0=c_ps[0:64, :], in1=bsb)
                            nc.sync.dma_start(
                                out=ctxd.ap()[hp * 128 + row0:hp * 128 + row0 + 64,
                                              qs], in_=cn)

            # ---- Phase 3: output projection (out [tq, o]) ----
            if "3" in phases:
                ctx_r = ctxd.ap().rearrange("(c p) t -> p c t", p=128)  # [128,8,TQ]
                with tc.tile_pool(name="wo", bufs=1) as wp, \
                     tc.tile_pool(name="ctxin", bufs=3) as cip, \
                     tc.tile_pool(name="osb", bufs=4) as op, \
                     tc.tile_pool(name="ps3", bufs=4, space="PSUM") as ps:
                    wo = [wp.tile([128, D], F32R, name=f"wo{i}", tag=f"wo{i}")
                          for i in range(8)]
                    bo = wp.tile([1, D], F32R, tag="bo")
                    nc.sync.dma_start(out=bo, in_=bod.ap())
                    for i in range(8):
                        nc.sync.dma_start(out=wo[i],
                                          in_=woT.ap()[i * 128:(i + 1) * 128, :])
                    for qt in range(8):
                        ctxt = cip.tile([128, 8, 128], F32R, tag="ctxt")
                        nc.sync.dma_start(
                            out=ctxt, in_=ctx_r[:, :, qt * 128:(qt + 1) * 128])
                        for nb in range(2):
                            p = ps.tile([128, 512], F32)
                            for hp in range(8):
                                nc.tensor.matmul(
                                    p, ctxt[:, hp, :],
                                    wo[hp][:, nb * 512:(nb + 1) * 512],
                                    start=(hp == 0), stop=False)
                            nc.tensor.matmul(p, ones[:, 0:128],
                                             bo[:, nb * 512:(nb + 1) * 512],
                                             start=False, stop=True)
                            osb = op.tile([128, 512], F32, tag="osb")
                            nc.vector.tensor_copy(out=osb, in_=p)
                            nc.sync.dma_start(
                                out=outd.ap()[qt * 128:(qt + 1) * 128,
                                              nb * 512:(nb + 1) * 512],
                                in_=osb)
            if "c" in phases:
                vp.release()
            if "b" in phases:
                kp.release()

    nc.compile()
    return nc


def get_nc(loop_n=1, phases="abc23"):
    key = ("nc", loop_n, phases)
    if key not in _CACHE:
        _CACHE[key] = _build(loop_n, phases)
    return _CACHE[key]


def make_in_maps(query, key, value, mask, Wq, bq, Wk, bk, Wv, bv, Wo, bo):
    query = np.asarray(query, dtype=np.float32)
    key = np.asarray(key, dtype=np.float32)
    value = np.asarray(value, dtype=np.float32)
    mask = np.asarray(mask)
    wqT = np.ascontiguousarray(np.asarray(Wq, np.float32).T)
    wkT = np.ascontiguousarray(np.asarray(Wk, np.float32).T)
    wvT = np.ascontiguousarray(np.asarray(Wv, np.float32).T)
    woT = np.ascontiguousarray(np.asarray(Wo, np.float32).T)
    bq_col = np.ascontiguousarray(np.asarray(bq, np.float32).reshape(8, 128).T)
    bk_col = np.ascontiguousarray(np.asarray(bk, np.float32).reshape(8, 128).T)
    bvr = np.asarray(bv, np.float32).reshape(1, D)
    bor = np.asarray(bo, np.float32).reshape(1, D)

    in_maps = []
    for c in range(NCORES):
        b, half = divmod(c, 2)
        t0 = half * TQ
        mask_add = np.where(mask[b, 0, 0, :] == 0, NEG, 0.0).astype(np.float32)
        in_maps.append({
            "xqT": np.ascontiguousarray(query[b, t0:t0 + TQ, :].T),
            "xkT": np.ascontiguousarray(key[b].T),
            "xvT": np.ascontiguousarray(value[b].T),
            "wqT": wqT, "wkT": wkT, "wvT": wvT, "woT": woT,
            "bq_col": bq_col, "bk_col": bk_col, "bv": bvr, "bo": bor,
            "mask_add": np.ascontiguousarray(mask_add.reshape(S // 128, 128).T),
            "ones_row": np.ones((1, 512), np.float32),
            "ones_col": np.ones((128, S // 128), np.float32),
        })
    return in_maps


def assemble(results):
    out = np.empty((B, S, D), np.float32)
    for c, r in enumerate(results):
        b, half = divmod(c, 2)
        out[b, half * TQ:half * TQ + TQ, :] = r["out"]
    return out


def kernel(**inputs):
    nc = get_nc()
    in_maps = make_in_maps(**inputs)
    res = run_bass_kernel_spmd(nc, in_maps, core_ids=list(range(NCORES)))
    return assemble(res.results)
